# revision 2
# baseline (speedup 1.0000x reference)
"""2-layer GRU (T=512, B=64, E=300, H=512) on 8 NeuronCores.

Strategy: data-parallel over batch (8 seqs/core), zero collectives.
Per core, transposed layouts [feature-on-partition, (time,batch)]:
  - embedding gather via indirect DMA, PE-transpose, batched input
    projections (bf16 matmuls, chunked 32 steps at a time)
  - serial recurrence: per layer-step 48 LDW+MM (Whh.T tiles as
    stationary operands, h.T as moving), gates on VE/ACT
  - layer 1 lags layer 0 by one 32-step chunk so its PE matmuls hide
    layer 0's gate latency (and vice versa)
Host: final 8-way partial-sum, mean, and the tiny FC.
"""
import numpy as np
import ml_dtypes

T, B, E, H, V, L = 512, 64, 300, 512, 30000, 5
NCORE = 8
BC = B // NCORE          # 8 sequences per core
CH = 32                  # steps per chunk
NCH = T // CH            # 16 chunks
SEG = (T + 1) * BC       # 4104 cols per H-chunk segment in state buffers
G3 = 3 * H               # 1536
KH = H // 128            # 4
M3 = G3 // 128           # 12
NBLK = T * BC // 128     # 32 gather blocks of 128 tokens
CB = CH * BC             # 256 cols per chunk

_cache = {}


def _build():
    from contextlib import ExitStack
    import concourse.bass as bass
    import concourse.mybir as mybir
    import concourse.tile as tile
    from concourse import bacc
    from concourse.masks import make_identity

    bf16, f32, i32 = mybir.dt.bfloat16, mybir.dt.float32, mybir.dt.int32
    AF = mybir.ActivationFunctionType
    OP = mybir.AluOpType

    nc = bacc.Bacc("TRN2", target_bir_lowering=False, debug=False,
                   num_devices=NCORE)
    emb_d = nc.dram_tensor("emb", [V, E], f32, kind="ExternalInput").ap()
    idx_d = nc.dram_tensor("idx", [128, NBLK], i32, kind="ExternalInput").ap()
    w0i_d = nc.dram_tensor("w0i", [128, 3 * G3], bf16, kind="ExternalInput").ap()
    w0h_d = nc.dram_tensor("w0h", [128, KH * G3], bf16, kind="ExternalInput").ap()
    w1i_d = nc.dram_tensor("w1i", [128, KH * G3], bf16, kind="ExternalInput").ap()
    w1h_d = nc.dram_tensor("w1h", [128, KH * G3], bf16, kind="ExternalInput").ap()
    b0_d = nc.dram_tensor("b0", [128, M3], f32, kind="ExternalInput").ap()
    b1_d = nc.dram_tensor("b1", [128, M3], f32, kind="ExternalInput").ap()
    bn0_d = nc.dram_tensor("bn0", [128, KH * BC], f32, kind="ExternalInput").ap()
    bn1_d = nc.dram_tensor("bn1", [128, KH * BC], f32, kind="ExternalInput").ap()
    out_d = nc.dram_tensor("out", [128, KH * T], f32, kind="ExternalOutput").ap()

    with tile.TileContext(nc) as tc, ExitStack() as ctx:
        wp = ctx.enter_context(tc.tile_pool(name="wp", bufs=1))
        sp = ctx.enter_context(tc.tile_pool(name="sp", bufs=1))
        xb = ctx.enter_context(tc.tile_pool(name="xb", bufs=2))
        tp = ctx.enter_context(tc.tile_pool(name="tp", bufs=3))
        pp = ctx.enter_context(tc.tile_pool(name="pp", bufs=2, space="PSUM"))
        px = ctx.enter_context(tc.tile_pool(name="px", bufs=2, space="PSUM"))

        def wtile(nm, shape, dt):
            t = wp.tile(shape, dt, name=nm, tag=nm)
            return t

        w0i = wtile("w0i_t", [128, 3 * G3], bf16)
        nc.sync.dma_start(out=w0i[:], in_=w0i_d[:])
        w0h = wtile("w0h_t", [128, KH * G3], bf16)
        nc.sync.dma_start(out=w0h[:], in_=w0h_d[:])
        w1i = wtile("w1i_t", [128, KH * G3], bf16)
        nc.sync.dma_start(out=w1i[:], in_=w1i_d[:])
        w1h = wtile("w1h_t", [128, KH * G3], bf16)
        nc.sync.dma_start(out=w1h[:], in_=w1h_d[:])
        b0 = wtile("b0_t", [128, M3], f32)
        nc.sync.dma_start(out=b0[:], in_=b0_d[:])
        b1 = wtile("b1_t", [128, M3], f32)
        nc.sync.dma_start(out=b1[:], in_=b1_d[:])
        bn0 = wtile("bn0_t", [128, KH * BC], f32)
        nc.sync.dma_start(out=bn0[:], in_=bn0_d[:])
        bn1 = wtile("bn1_t", [128, KH * BC], f32)
        nc.sync.dma_start(out=bn1[:], in_=bn1_d[:])
        idx_t = wtile("idx_t", [128, NBLK], i32)
        nc.sync.dma_start(out=idx_t[:], in_=idx_d[:])
        ident = wtile("ident", [128, 128], bf16)
        make_identity(nc, ident[:])

        st0 = sp.tile([128, KH * SEG], bf16, name="st0", tag="st0")
        st1 = sp.tile([128, KH * SEG], bf16, name="st1", tag="st1")
        pooled = sp.tile([128, KH * T], f32, name="pooled", tag="pooled")
        for k in range(KH):
            nc.vector.memset(st0[:, k * SEG:k * SEG + BC], 0.0)
            nc.vector.memset(st1[:, k * SEG:k * SEG + BC], 0.0)

        bn0v = bn0[:].rearrange("p (c b) -> p c b", c=KH)
        bn1v = bn1[:].rearrange("p (c b) -> p c b", c=KH)

        xp0bufs = {}
        xp1bufs = {}

        def stage_x0(c):
            """gather + transpose + input projection for chunk c of layer 0"""
            xT = []
            for e in range(3):
                xTe = xb.tile([128, 2 * 128], bf16, name=f"xT{e}", tag=f"xT{e}")
                xT.append(xTe)
            for g in range(2):
                blk = 2 * c + g
                xr = tp.tile([128, E], f32, name="xr", tag="xr")
                nc.gpsimd.indirect_dma_start(
                    out=xr[:], out_offset=None, in_=emb_d[:],
                    in_offset=bass.IndirectOffsetOnAxis(
                        ap=idx_t[:, blk:blk + 1], axis=0))
                xc = tp.tile([128, E], bf16, name="xc", tag="xc")
                nc.vector.tensor_copy(out=xc[:], in_=xr[:])
                for e in range(3):
                    ke = min(128, E - e * 128)
                    tps = px.tile([128, 128], bf16, name="tps", tag="tps")
                    nc.tensor.transpose(out=tps[0:ke, :],
                                        in_=xc[:, e * 128:e * 128 + ke],
                                        identity=ident[:])
                    nc.vector.tensor_copy(out=xT[e][0:ke, g * 128:(g + 1) * 128],
                                          in_=tps[0:ke, :])
            xpb = xb.tile([128, M3 * CB], bf16, name="xp0b", tag="xp0b")
            xp0bufs[c] = xpb
            for m in range(M3):
                xpp = px.tile([128, CB], f32, name="xpp", tag="xpp")
                for k in range(3):
                    ke = min(128, E - k * 128)
                    nc.tensor.matmul(
                        out=xpp[:, 0:CB],
                        lhsT=w0i[0:ke, k * G3 + m * 128:k * G3 + (m + 1) * 128],
                        rhs=xT[k][0:ke, 0:CB],
                        start=(k == 0), stop=(k == 2))
                nc.scalar.activation(out=xpb[:, m * CB:(m + 1) * CB],
                                     in_=xpp[:, 0:CB], func=AF.Identity,
                                     bias=b0[:, m:m + 1])

        def stage_x1(c):
            """input projection for chunk c of layer 1 (from st0 cols)"""
            xpb = xb.tile([128, M3 * CB], bf16, name="xp1b", tag="xp1b")
            xp1bufs[c] = xpb
            for m in range(M3):
                xpp = px.tile([128, CB], f32, name="xpp", tag="xpp")
                for k in range(KH):
                    nc.tensor.matmul(
                        out=xpp[:, 0:CB],
                        lhsT=w1i[:, k * G3 + m * 128:k * G3 + (m + 1) * 128],
                        rhs=st0[:, k * SEG + (c * CH + 1) * BC:
                                k * SEG + (c * CH + 1) * BC + CB],
                        start=(k == 0), stop=(k == KH - 1))
                nc.scalar.activation(out=xpb[:, m * CB:(m + 1) * CB],
                                     in_=xpp[:, 0:CB], func=AF.Identity,
                                     bias=b1[:, m:m + 1])

        def step(layer, t):
            st = st0 if layer == 0 else st1
            w = w0h if layer == 0 else w1h
            bnv = bn0v if layer == 0 else bn1v
            c = t // CH
            ts = t % CH
            xpb = (xp0bufs if layer == 0 else xp1bufs)[c]
            xpv = xpb[:].rearrange("p (m s) -> p m s", m=M3)
            gh = pp.tile([128, M3 * BC], f32, name=f"gh{layer}",
                         tag=f"gh{layer}")
            for m in range(M3):
                for k in range(KH):
                    nc.tensor.matmul(
                        out=gh[:, m * BC:(m + 1) * BC],
                        lhsT=w[:, k * G3 + m * 128:k * G3 + (m + 1) * 128],
                        rhs=st[:, k * SEG + t * BC:k * SEG + (t + 1) * BC],
                        start=(k == 0), stop=(k == KH - 1))
            ghv = gh[:].rearrange("p (m b) -> p m b", b=BC)
            sl = ts * BC

            def tmp(nm):
                tt = tp.tile([128, KH * BC], f32, name=f"{nm}{layer}",
                             tag=f"{nm}{layer}")
                return tt, tt[:].rearrange("p (c b) -> p c b", c=KH)

            ar, arv = tmp("ar")
            nc.vector.tensor_tensor(out=arv, in0=xpv[:, 0:4, sl:sl + BC],
                                    in1=ghv[:, 0:4, :], op=OP.add)
            r, _ = tmp("r")
            nc.scalar.activation(out=r[:], in_=ar[:], func=AF.Sigmoid)
            az, azv = tmp("az")
            nc.vector.tensor_tensor(out=azv, in0=xpv[:, 4:8, sl:sl + BC],
                                    in1=ghv[:, 4:8, :], op=OP.add)
            z, _ = tmp("z")
            nc.scalar.activation(out=z[:], in_=az[:], func=AF.Sigmoid)
            hn, hnv = tmp("hn")
            nc.vector.tensor_tensor(out=hnv, in0=ghv[:, 8:12, :], in1=bnv,
                                    op=OP.add)
            hm, _ = tmp("hm")
            nc.vector.tensor_tensor(out=hm[:], in0=r[:], in1=hn[:], op=OP.mult)
            an, anv = tmp("an")
            nc.vector.tensor_tensor(out=anv, in0=xpv[:, 8:12, sl:sl + BC],
                                    in1=hm[:].rearrange("p (c b) -> p c b", c=KH),
                                    op=OP.add)
            n, _ = tmp("n")
            nc.scalar.activation(out=n[:], in_=an[:], func=AF.Tanh)
            stv = st[:].rearrange("p (c s) -> p c s", c=KH)
            d, dv = tmp("d")
            nc.vector.tensor_tensor(out=dv, in0=stv[:, :, t * BC:(t + 1) * BC],
                                    in1=n[:].rearrange("p (c b) -> p c b", c=KH),
                                    op=OP.subtract)
            e_, _ = tmp("e")
            nc.vector.tensor_tensor(out=e_[:], in0=z[:], in1=d[:], op=OP.mult)
            nc.vector.tensor_tensor(
                out=stv[:, :, (t + 1) * BC:(t + 2) * BC],
                in0=n[:].rearrange("p (c b) -> p c b", c=KH),
                in1=e_[:].rearrange("p (c b) -> p c b", c=KH), op=OP.add)

        stage_x0(0)
        for c in range(NCH):
            if c + 1 < NCH:
                stage_x0(c + 1)
            if c >= 1:
                stage_x1(c - 1)
            for ts in range(CH):
                step(0, c * CH + ts)
                if c >= 1:
                    step(1, (c - 1) * CH + ts)
        stage_x1(NCH - 1)
        for ts in range(CH):
            step(1, (NCH - 1) * CH + ts)

        nc.vector.tensor_reduce(
            out=pooled[:].rearrange("p (c t) -> p c t", c=KH),
            in_=st1[:].rearrange("p (c s b) -> p c s b", c=KH, b=BC)[:, :, 1:T + 1, :],
            axis=mybir.AxisListType.X, op=OP.add)
        nc.sync.dma_start(out=out_d[:], in_=pooled[:])

    nc.compile()
    return nc


def _prep(inputs):
    bf = ml_dtypes.bfloat16

    def packT(W):
        WT = np.ascontiguousarray(W.T).astype(np.float32)
        K = WT.shape[0]
        nk = (K + 127) // 128
        pad = np.zeros((nk * 128, G3), np.float32)
        pad[:K] = WT
        return np.concatenate([pad[k * 128:(k + 1) * 128] for k in range(nk)],
                              axis=1).astype(bf)

    def bias_rz(bih, bhh):
        b = bih.astype(np.float32).copy()
        b[:2 * H] += bhh[:2 * H].astype(np.float32)
        return np.ascontiguousarray(b.reshape(M3, 128).T)

    def bias_n(bhh):
        bn = np.ascontiguousarray(
            bhh[2 * H:].astype(np.float32).reshape(KH, 128).T)
        return np.repeat(bn, BC, axis=1)

    common = {
        "emb": np.ascontiguousarray(inputs["emb"], dtype=np.float32),
        "w0i": packT(inputs["Wih0"]),
        "w0h": packT(inputs["Whh0"]),
        "w1i": packT(inputs["Wih1"]),
        "w1h": packT(inputs["Whh1"]),
        "b0": bias_rz(inputs["bih0"], inputs["bhh0"]),
        "b1": bias_rz(inputs["bih1"], inputs["bhh1"]),
        "bn0": bias_n(inputs["bhh0"]),
        "bn1": bias_n(inputs["bhh1"]),
    }
    texts = np.asarray(inputs["texts"])
    in_maps = []
    for c in range(NCORE):
        idxc = np.ascontiguousarray(
            texts[:, c * BC:(c + 1) * BC].astype(np.int32)
            .reshape(NBLK, 128).T)
        in_maps.append({**common, "idx": idxc})
    return in_maps


def _postproc(results, inputs):
    s = np.zeros((128, KH * T), np.float32)
    for r in results:
        s += r["out"]
    pooled = s.reshape(128, KH, T).transpose(2, 1, 0).reshape(T, H) / B
    fc_W = np.asarray(inputs["fc_W"], dtype=np.float32)
    fc_b = np.asarray(inputs["fc_b"], dtype=np.float32)
    return (pooled @ fc_W.T + fc_b).astype(np.float32)


def kernel(**inputs):
    from concourse import bass_utils
    if "nc" not in _cache:
        _cache["nc"] = _build()
    nc = _cache["nc"]
    in_maps = _prep(inputs)
    res = bass_utils.run_bass_kernel_spmd(
        nc, in_maps, core_ids=list(range(NCORE)))
    return _postproc([res.results[i] for i in range(NCORE)], inputs)


if __name__ == "__main__":
    import time
    t0 = time.time()
    nc = _build()
    print("build+compile time:", round(time.time() - t0, 1), "s")


# revision 8
# speedup vs baseline: 3675.7335x; 3675.7335x over previous
"""2-layer GRU (T=512, B=64, E=300, H=512) on 8 NeuronCores.

Strategy: data-parallel over batch (8 seqs/core), zero collectives.
Per core, transposed layouts [feature-on-partition, (time,batch)]:
  - embedding gather via indirect DMA, PE-transpose, batched input
    projections (bf16 matmuls, chunked 32 steps at a time)
  - serial recurrence: per layer-step 48 LDW+MM (Whh.T tiles as
    stationary operands, h.T as moving), gates on VE/ACT
  - layer 1 lags layer 0 by one 32-step chunk so its PE matmuls hide
    layer 0's gate latency (and vice versa)
Host: final 8-way partial-sum, mean, and the tiny FC.
"""
import numpy as np
import ml_dtypes

T, B, E, H, V, L = 512, 64, 300, 512, 30000, 5
NCORE = 8
BC = B // NCORE          # 8 sequences per core
CH = 32                  # steps per chunk
NCH = T // CH            # 16 chunks
SEG = (T + 1) * BC       # 4104 cols per H-chunk segment in state buffers
G3 = 3 * H               # 1536
KH = H // 128            # 4
M3 = G3 // 128           # 12
NBLK = T * BC // 128     # 32 gather blocks of 128 tokens
CB = CH * BC             # 256 cols per chunk

_cache = {}


def _build():
    from contextlib import ExitStack
    import concourse.bass as bass
    import concourse.mybir as mybir
    import concourse.tile as tile
    from concourse import bacc
    from concourse.masks import make_identity

    bf16, f32, i32 = mybir.dt.bfloat16, mybir.dt.float32, mybir.dt.int32
    AF = mybir.ActivationFunctionType
    OP = mybir.AluOpType

    nc = bacc.Bacc("TRN2", target_bir_lowering=False, debug=False,
                   num_devices=NCORE)
    emb_d = nc.dram_tensor("emb", [V, E], f32, kind="ExternalInput").ap()
    idx_d = nc.dram_tensor("idx", [128, NBLK], i32, kind="ExternalInput").ap()
    w0i_d = nc.dram_tensor("w0i", [128, 3 * G3], bf16, kind="ExternalInput").ap()
    w0h_d = nc.dram_tensor("w0h", [128, KH * G3], bf16, kind="ExternalInput").ap()
    w1i_d = nc.dram_tensor("w1i", [128, KH * G3], bf16, kind="ExternalInput").ap()
    w1h_d = nc.dram_tensor("w1h", [128, KH * G3], bf16, kind="ExternalInput").ap()
    b0_d = nc.dram_tensor("b0", [128, M3], f32, kind="ExternalInput").ap()
    b1_d = nc.dram_tensor("b1", [128, M3], f32, kind="ExternalInput").ap()
    bn0_d = nc.dram_tensor("bn0", [128, KH * BC], f32, kind="ExternalInput").ap()
    bn1_d = nc.dram_tensor("bn1", [128, KH * BC], f32, kind="ExternalInput").ap()
    out_d = nc.dram_tensor("out", [128, KH * T], f32, kind="ExternalOutput").ap()

    with tile.TileContext(nc) as tc, ExitStack() as ctx:
        wp = ctx.enter_context(tc.tile_pool(name="wp", bufs=1))
        sp = ctx.enter_context(tc.tile_pool(name="sp", bufs=1))
        xb = ctx.enter_context(tc.tile_pool(name="xb", bufs=2))
        tp = ctx.enter_context(tc.tile_pool(name="tp", bufs=3))
        pp = ctx.enter_context(tc.tile_pool(name="pp", bufs=2, space="PSUM"))
        px = ctx.enter_context(tc.tile_pool(name="px", bufs=2, space="PSUM"))

        def wtile(nm, shape, dt):
            t = wp.tile(shape, dt, name=nm, tag=nm)
            return t

        w0i = wtile("w0i_t", [128, 3 * G3], bf16)
        nc.sync.dma_start(out=w0i[:], in_=w0i_d[:])
        w0h = wtile("w0h_t", [128, KH * G3], bf16)
        nc.sync.dma_start(out=w0h[:], in_=w0h_d[:])
        w1i = wtile("w1i_t", [128, KH * G3], bf16)
        nc.sync.dma_start(out=w1i[:], in_=w1i_d[:])
        w1h = wtile("w1h_t", [128, KH * G3], bf16)
        nc.sync.dma_start(out=w1h[:], in_=w1h_d[:])
        b0 = wtile("b0_t", [128, M3], f32)
        nc.sync.dma_start(out=b0[:], in_=b0_d[:])
        b1 = wtile("b1_t", [128, M3], f32)
        nc.sync.dma_start(out=b1[:], in_=b1_d[:])
        bn0 = wtile("bn0_t", [128, KH * BC], f32)
        nc.sync.dma_start(out=bn0[:], in_=bn0_d[:])
        bn1 = wtile("bn1_t", [128, KH * BC], f32)
        nc.sync.dma_start(out=bn1[:], in_=bn1_d[:])
        idx_t = wtile("idx_t", [128, NBLK], i32)
        nc.sync.dma_start(out=idx_t[:], in_=idx_d[:])
        ident = wtile("ident", [128, 128], bf16)
        make_identity(nc, ident[:])

        st0 = sp.tile([128, KH * SEG], bf16, name="st0", tag="st0")
        st1 = sp.tile([128, KH * SEG], bf16, name="st1", tag="st1")
        pooled = sp.tile([128, KH * T], f32, name="pooled", tag="pooled")
        for k in range(KH):
            nc.vector.memset(st0[:, k * SEG:k * SEG + BC], 0.0)
            nc.vector.memset(st1[:, k * SEG:k * SEG + BC], 0.0)



        xp0bufs = {}
        xp1bufs = {}

        def stage_x0(c):
            """gather + transpose + input projection for chunk c of layer 0"""
            xT = []
            for e in range(3):
                xTe = xb.tile([128, 2 * 128], bf16, name=f"xT{e}", tag=f"xT{e}")
                xT.append(xTe)
            for g in range(2):
                blk = 2 * c + g
                xr = tp.tile([128, E], f32, name="xr", tag="xr")
                nc.gpsimd.indirect_dma_start(
                    out=xr[:], out_offset=None, in_=emb_d[:],
                    in_offset=bass.IndirectOffsetOnAxis(
                        ap=idx_t[:, blk:blk + 1], axis=0))
                xc = tp.tile([128, E], bf16, name="xc", tag="xc")
                nc.vector.tensor_copy(out=xc[:], in_=xr[:])
                for e in range(3):
                    ke = min(128, E - e * 128)
                    tps = px.tile([128, 128], bf16, name="tps", tag="tps")
                    nc.tensor.transpose(out=tps[0:ke, :],
                                        in_=xc[:, e * 128:e * 128 + ke],
                                        identity=ident[:])
                    nc.vector.tensor_copy(out=xT[e][0:ke, g * 128:(g + 1) * 128],
                                          in_=tps[0:ke, :])
            xpb = xb.tile([128, M3 * CB], bf16, name="xp0b", tag="xp0b")
            xp0bufs[c] = xpb
            for m in range(M3):
                xpp = px.tile([128, CB], f32, name="xpp", tag="xpp")
                for k in range(3):
                    ke = min(128, E - k * 128)
                    nc.tensor.matmul(
                        out=xpp[:, 0:CB],
                        lhsT=w0i[0:ke, k * G3 + m * 128:k * G3 + (m + 1) * 128],
                        rhs=xT[k][0:ke, 0:CB],
                        start=(k == 0), stop=(k == 2))
                nc.scalar.activation(out=xpb[:, m * CB:(m + 1) * CB],
                                     in_=xpp[:, 0:CB], func=AF.Identity,
                                     bias=b0[:, m:m + 1])

        def stage_x1(c):
            """input projection for chunk c of layer 1 (from st0 cols)"""
            xpb = xb.tile([128, M3 * CB], bf16, name="xp1b", tag="xp1b")
            xp1bufs[c] = xpb
            for m in range(M3):
                xpp = px.tile([128, CB], f32, name="xpp", tag="xpp")
                for k in range(KH):
                    nc.tensor.matmul(
                        out=xpp[:, 0:CB],
                        lhsT=w1i[:, k * G3 + m * 128:k * G3 + (m + 1) * 128],
                        rhs=st0[:, k * SEG + (c * CH + 1) * BC:
                                k * SEG + (c * CH + 1) * BC + CB],
                        start=(k == 0), stop=(k == KH - 1))
                nc.scalar.activation(out=xpb[:, m * CB:(m + 1) * CB],
                                     in_=xpp[:, 0:CB], func=AF.Identity,
                                     bias=b1[:, m:m + 1])

        def step(layer, t):
            st = st0 if layer == 0 else st1
            w = w0h if layer == 0 else w1h
            bn = bn0 if layer == 0 else bn1
            c = t // CH
            ts = t % CH
            xpb = (xp0bufs if layer == 0 else xp1bufs)[c]
            xpv = xpb[:].rearrange("p (m s) -> p m s", m=M3)
            gh = pp.tile([128, M3 * BC], f32, name=f"gh{layer}",
                         tag=f"gh{layer}")
            for m in range(M3):
                for k in range(KH):
                    nc.tensor.matmul(
                        out=gh[:, m * BC:(m + 1) * BC],
                        lhsT=w[:, k * G3 + m * 128:k * G3 + (m + 1) * 128],
                        rhs=st[:, k * SEG + t * BC:k * SEG + (t + 1) * BC],
                        start=(k == 0), stop=(k == KH - 1))
            ghv = gh[:].rearrange("p (m b) -> p m b", b=BC)
            sl = ts * BC

            def tmp(nm, width=KH * BC):
                tt = tp.tile([128, width], f32, name=f"{nm}{layer}",
                             tag=f"{nm}{layer}")
                return tt

            arz = tmp("arz", 2 * KH * BC)
            nc.vector.tensor_tensor(
                out=arz[:].rearrange("p (c b) -> p c b", c=2 * KH),
                in0=xpv[:, 0:8, sl:sl + BC], in1=ghv[:, 0:8, :], op=OP.add)
            rz = tmp("rz", 2 * KH * BC)
            nc.scalar.activation(out=rz[:], in_=arz[:], func=AF.Sigmoid)
            hn = tmp("hn")
            nc.vector.tensor_tensor(
                out=hn[:].rearrange("p (c b) -> p c b", c=KH),
                in0=ghv[:, 8:12, :],
                in1=bn[:].rearrange("p (c b) -> p c b", c=KH), op=OP.add)
            hm = tmp("hm")
            nc.vector.tensor_tensor(out=hm[:], in0=rz[:, 0:KH * BC],
                                    in1=hn[:], op=OP.mult)
            an = tmp("an")
            nc.vector.tensor_tensor(
                out=an[:].rearrange("p (c b) -> p c b", c=KH),
                in0=xpv[:, 8:12, sl:sl + BC],
                in1=hm[:].rearrange("p (c b) -> p c b", c=KH), op=OP.add)
            n = tmp("n")
            nc.scalar.activation(out=n[:], in_=an[:], func=AF.Tanh)
            stv = st[:].rearrange("p (c s) -> p c s", c=KH)
            d = tmp("d")
            nc.vector.tensor_tensor(
                out=d[:].rearrange("p (c b) -> p c b", c=KH),
                in0=stv[:, :, t * BC:(t + 1) * BC],
                in1=n[:].rearrange("p (c b) -> p c b", c=KH),
                op=OP.subtract)
            e_ = tmp("e")
            nc.vector.tensor_tensor(out=e_[:], in0=rz[:, KH * BC:2 * KH * BC],
                                    in1=d[:], op=OP.mult)
            nc.vector.tensor_tensor(
                out=stv[:, :, (t + 1) * BC:(t + 2) * BC],
                in0=n[:].rearrange("p (c b) -> p c b", c=KH),
                in1=e_[:].rearrange("p (c b) -> p c b", c=KH), op=OP.add)

        stage_x0(0)
        for c in range(NCH):
            if c + 1 < NCH:
                stage_x0(c + 1)
            if c >= 1:
                stage_x1(c - 1)
            for ts in range(CH):
                step(0, c * CH + ts)
                if c >= 1:
                    step(1, (c - 1) * CH + ts)
        stage_x1(NCH - 1)
        for ts in range(CH):
            step(1, (NCH - 1) * CH + ts)

        nc.vector.tensor_reduce(
            out=pooled[:].rearrange("p (c t) -> p c t", c=KH),
            in_=st1[:].rearrange("p (c s b) -> p c s b", c=KH, b=BC)[:, :, 1:T + 1, :],
            axis=mybir.AxisListType.X, op=OP.add)
        nc.sync.dma_start(out=out_d[:], in_=pooled[:])

    nc.compile()
    return nc


def _prep(inputs):
    bf = ml_dtypes.bfloat16

    def packT(W):
        WT = np.ascontiguousarray(W.T).astype(np.float32)
        K = WT.shape[0]
        nk = (K + 127) // 128
        pad = np.zeros((nk * 128, G3), np.float32)
        pad[:K] = WT
        return np.concatenate([pad[k * 128:(k + 1) * 128] for k in range(nk)],
                              axis=1).astype(bf)

    def bias_rz(bih, bhh):
        b = bih.astype(np.float32).copy()
        b[:2 * H] += bhh[:2 * H].astype(np.float32)
        return np.ascontiguousarray(b.reshape(M3, 128).T)

    def bias_n(bhh):
        bn = np.ascontiguousarray(
            bhh[2 * H:].astype(np.float32).reshape(KH, 128).T)
        return np.repeat(bn, BC, axis=1)

    common = {
        "emb": np.ascontiguousarray(inputs["emb"], dtype=np.float32),
        "w0i": packT(inputs["Wih0"]),
        "w0h": packT(inputs["Whh0"]),
        "w1i": packT(inputs["Wih1"]),
        "w1h": packT(inputs["Whh1"]),
        "b0": bias_rz(inputs["bih0"], inputs["bhh0"]),
        "b1": bias_rz(inputs["bih1"], inputs["bhh1"]),
        "bn0": bias_n(inputs["bhh0"]),
        "bn1": bias_n(inputs["bhh1"]),
    }
    texts = np.asarray(inputs["texts"])
    in_maps = []
    for c in range(NCORE):
        idxc = np.ascontiguousarray(
            texts[:, c * BC:(c + 1) * BC].astype(np.int32)
            .reshape(NBLK, 128).T)
        in_maps.append({**common, "idx": idxc})
    return in_maps


def _postproc(results, inputs):
    s = np.zeros((128, KH * T), np.float32)
    for r in results:
        s += r["out"]
    pooled = s.reshape(128, KH, T).transpose(2, 1, 0).reshape(T, H) / B
    fc_W = np.asarray(inputs["fc_W"], dtype=np.float32)
    fc_b = np.asarray(inputs["fc_b"], dtype=np.float32)
    return (pooled @ fc_W.T + fc_b).astype(np.float32)


def kernel(**inputs):
    from concourse import bass_utils
    if "nc" not in _cache:
        _cache["nc"] = _build()
    nc = _cache["nc"]
    in_maps = _prep(inputs)
    res = bass_utils.run_bass_kernel_spmd(
        nc, in_maps, core_ids=list(range(NCORE)))
    return _postproc([res.results[i] for i in range(NCORE)], inputs)


if __name__ == "__main__":
    import time
    t0 = time.time()
    nc = _build()
    print("build+compile time:", round(time.time() - t0, 1), "s")


# revision 10
# speedup vs baseline: 3803.9925x; 1.0349x over previous
"""2-layer GRU (T=512, B=64, E=300, H=512) on 8 NeuronCores.

Strategy: data-parallel over batch (8 seqs/core), zero collectives.
Per core, transposed layouts [feature-on-partition, (time,batch)]:
  - embedding gather via indirect DMA, PE-transpose, batched input
    projections (bf16 matmuls, chunked 32 steps at a time)
  - serial recurrence: per layer-step 48 LDW+MM (Whh.T tiles as
    stationary operands, h.T as moving), gates on VE/ACT
  - layer 1 lags layer 0 by one 32-step chunk so its PE matmuls hide
    layer 0's gate latency (and vice versa)
Host: final 8-way partial-sum, mean, and the tiny FC.
"""
import numpy as np
import ml_dtypes

T, B, E, H, V, L = 512, 64, 300, 512, 30000, 5
NCORE = 8
BC = B // NCORE          # 8 sequences per core
CH = 32                  # steps per chunk
NCH = T // CH            # 16 chunks
SEG = (T + 1) * BC       # 4104 cols per H-chunk segment in state buffers
G3 = 3 * H               # 1536
KH = H // 128            # 4
M3 = G3 // 128           # 12
NBLK = T * BC // 128     # 32 gather blocks of 128 tokens
CB = CH * BC             # 256 cols per chunk

_cache = {}


def _build():
    from contextlib import ExitStack
    import concourse.bass as bass
    import concourse.mybir as mybir
    import concourse.tile as tile
    from concourse import bacc
    from concourse.masks import make_identity

    bf16, f32, i32 = mybir.dt.bfloat16, mybir.dt.float32, mybir.dt.int32
    AF = mybir.ActivationFunctionType
    OP = mybir.AluOpType

    nc = bacc.Bacc("TRN2", target_bir_lowering=False, debug=False,
                   num_devices=NCORE)
    emb_d = nc.dram_tensor("emb", [V, E], f32, kind="ExternalInput").ap()
    idx_d = nc.dram_tensor("idx", [128, NBLK], i32, kind="ExternalInput").ap()
    w0i_d = nc.dram_tensor("w0i", [128, 3 * G3], bf16, kind="ExternalInput").ap()
    w0h_d = nc.dram_tensor("w0h", [128, KH * G3], bf16, kind="ExternalInput").ap()
    w1i_d = nc.dram_tensor("w1i", [128, KH * G3], bf16, kind="ExternalInput").ap()
    w1h_d = nc.dram_tensor("w1h", [128, KH * G3], bf16, kind="ExternalInput").ap()
    b0_d = nc.dram_tensor("b0", [128, M3], f32, kind="ExternalInput").ap()
    b1_d = nc.dram_tensor("b1", [128, M3], f32, kind="ExternalInput").ap()
    bn0_d = nc.dram_tensor("bn0", [128, KH * BC], f32, kind="ExternalInput").ap()
    bn1_d = nc.dram_tensor("bn1", [128, KH * BC], f32, kind="ExternalInput").ap()
    out_d = nc.dram_tensor("out", [128, KH * T], f32, kind="ExternalOutput").ap()

    with tile.TileContext(nc) as tc, ExitStack() as ctx:
        wp = ctx.enter_context(tc.tile_pool(name="wp", bufs=1))
        sp = ctx.enter_context(tc.tile_pool(name="sp", bufs=1))
        xb = ctx.enter_context(tc.tile_pool(name="xb", bufs=2))
        tp = ctx.enter_context(tc.tile_pool(name="tp", bufs=3))
        pp = ctx.enter_context(tc.tile_pool(name="pp", bufs=2, space="PSUM"))
        px = ctx.enter_context(tc.tile_pool(name="px", bufs=2, space="PSUM"))

        def wtile(nm, shape, dt):
            t = wp.tile(shape, dt, name=nm, tag=nm)
            return t

        w0i = wtile("w0i_t", [128, 3 * G3], bf16)
        nc.sync.dma_start(out=w0i[:], in_=w0i_d[:])
        w0h = wtile("w0h_t", [128, KH * G3], bf16)
        nc.sync.dma_start(out=w0h[:], in_=w0h_d[:])
        w1i = wtile("w1i_t", [128, KH * G3], bf16)
        nc.sync.dma_start(out=w1i[:], in_=w1i_d[:])
        w1h = wtile("w1h_t", [128, KH * G3], bf16)
        nc.sync.dma_start(out=w1h[:], in_=w1h_d[:])
        b0 = wtile("b0_t", [128, M3], f32)
        nc.sync.dma_start(out=b0[:], in_=b0_d[:])
        b1 = wtile("b1_t", [128, M3], f32)
        nc.sync.dma_start(out=b1[:], in_=b1_d[:])
        bn0 = wtile("bn0_t", [128, KH * BC], f32)
        nc.sync.dma_start(out=bn0[:], in_=bn0_d[:])
        bn1 = wtile("bn1_t", [128, KH * BC], f32)
        nc.sync.dma_start(out=bn1[:], in_=bn1_d[:])
        idx_t = wtile("idx_t", [128, NBLK], i32)
        nc.sync.dma_start(out=idx_t[:], in_=idx_d[:])
        ident = wtile("ident", [128, 128], bf16)
        make_identity(nc, ident[:])

        st0 = sp.tile([128, KH * SEG], bf16, name="st0", tag="st0")
        st1 = sp.tile([128, KH * SEG], bf16, name="st1", tag="st1")
        pooled = sp.tile([128, KH * T], f32, name="pooled", tag="pooled")
        for k in range(KH):
            nc.vector.memset(st0[:, k * SEG:k * SEG + BC], 0.0)
            nc.vector.memset(st1[:, k * SEG:k * SEG + BC], 0.0)



        xp0bufs = {}
        xp1bufs = {}

        def stage_x0(c):
            """gather + transpose + input projection for chunk c of layer 0"""
            xT = []
            for e in range(3):
                xTe = xb.tile([128, 2 * 128], bf16, name=f"xT{e}", tag=f"xT{e}")
                xT.append(xTe)
            for g in range(2):
                blk = 2 * c + g
                xr = tp.tile([128, E], f32, name="xr", tag="xr")
                nc.gpsimd.indirect_dma_start(
                    out=xr[:], out_offset=None, in_=emb_d[:],
                    in_offset=bass.IndirectOffsetOnAxis(
                        ap=idx_t[:, blk:blk + 1], axis=0))
                xc = tp.tile([128, E], bf16, name="xc", tag="xc")
                nc.vector.tensor_copy(out=xc[:], in_=xr[:])
                for e in range(3):
                    ke = min(128, E - e * 128)
                    tps = px.tile([128, 128], bf16, name="tps", tag="tps")
                    nc.tensor.transpose(out=tps[0:ke, :],
                                        in_=xc[:, e * 128:e * 128 + ke],
                                        identity=ident[:])
                    nc.vector.tensor_copy(out=xT[e][0:ke, g * 128:(g + 1) * 128],
                                          in_=tps[0:ke, :])
            xpb = xb.tile([128, M3 * CB], bf16, name="xp0b", tag="xp0b")
            xp0bufs[c] = xpb
            thunks = []

            def mk(m):
                def emit():
                    xpp = px.tile([128, CB], f32, name="xpp", tag="xpp")
                    for k in range(3):
                        ke = min(128, E - k * 128)
                        nc.tensor.matmul(
                            out=xpp[:, 0:CB],
                            lhsT=w0i[0:ke, k * G3 + m * 128:k * G3 + (m + 1) * 128],
                            rhs=xT[k][0:ke, 0:CB],
                            start=(k == 0), stop=(k == 2))
                    nc.scalar.activation(out=xpb[:, m * CB:(m + 1) * CB],
                                         in_=xpp[:, 0:CB], func=AF.Identity,
                                         bias=b0[:, m:m + 1])
                return emit

            for m in range(M3):
                thunks.append(mk(m))
            return thunks

        def stage_x1(c):
            """input projection for chunk c of layer 1 (from st0 cols)"""
            xpb = xb.tile([128, M3 * CB], bf16, name="xp1b", tag="xp1b")
            xp1bufs[c] = xpb
            for m in range(M3):
                xpp = px.tile([128, CB], f32, name="xpp", tag="xpp")
                for k in range(KH):
                    nc.tensor.matmul(
                        out=xpp[:, 0:CB],
                        lhsT=w1i[:, k * G3 + m * 128:k * G3 + (m + 1) * 128],
                        rhs=st0[:, k * SEG + (c * CH + 1) * BC:
                                k * SEG + (c * CH + 1) * BC + CB],
                        start=(k == 0), stop=(k == KH - 1))
                nc.scalar.activation(out=xpb[:, m * CB:(m + 1) * CB],
                                     in_=xpp[:, 0:CB], func=AF.Identity,
                                     bias=b1[:, m:m + 1])

        def step(layer, t):
            st = st0 if layer == 0 else st1
            w = w0h if layer == 0 else w1h
            bn = bn0 if layer == 0 else bn1
            c = t // CH
            ts = t % CH
            xpb = (xp0bufs if layer == 0 else xp1bufs)[c]
            xpv = xpb[:].rearrange("p (m s) -> p m s", m=M3)
            gh = pp.tile([128, M3 * BC], f32, name=f"gh{layer}",
                         tag=f"gh{layer}")
            for m in range(M3):
                for k in range(KH):
                    nc.tensor.matmul(
                        out=gh[:, m * BC:(m + 1) * BC],
                        lhsT=w[:, k * G3 + m * 128:k * G3 + (m + 1) * 128],
                        rhs=st[:, k * SEG + t * BC:k * SEG + (t + 1) * BC],
                        start=(k == 0), stop=(k == KH - 1))
            ghv = gh[:].rearrange("p (m b) -> p m b", b=BC)
            sl = ts * BC

            def tmp(nm, width=KH * BC):
                tt = tp.tile([128, width], f32, name=f"{nm}{layer}",
                             tag=f"{nm}{layer}")
                return tt

            arz = tmp("arz", 2 * KH * BC)
            nc.vector.tensor_tensor(
                out=arz[:].rearrange("p (c b) -> p c b", c=2 * KH),
                in0=xpv[:, 0:8, sl:sl + BC], in1=ghv[:, 0:8, :], op=OP.add)
            rz = tmp("rz", 2 * KH * BC)
            nc.scalar.activation(out=rz[:], in_=arz[:], func=AF.Sigmoid)
            hn = tmp("hn")
            nc.vector.tensor_tensor(
                out=hn[:].rearrange("p (c b) -> p c b", c=KH),
                in0=ghv[:, 8:12, :],
                in1=bn[:].rearrange("p (c b) -> p c b", c=KH), op=OP.add)
            hm = tmp("hm")
            nc.vector.tensor_tensor(out=hm[:], in0=rz[:, 0:KH * BC],
                                    in1=hn[:], op=OP.mult)
            an = tmp("an")
            nc.vector.tensor_tensor(
                out=an[:].rearrange("p (c b) -> p c b", c=KH),
                in0=xpv[:, 8:12, sl:sl + BC],
                in1=hm[:].rearrange("p (c b) -> p c b", c=KH), op=OP.add)
            n = tmp("n")
            nc.scalar.activation(out=n[:], in_=an[:], func=AF.Tanh)
            stv = st[:].rearrange("p (c s) -> p c s", c=KH)
            d = tmp("d")
            nc.vector.tensor_tensor(
                out=d[:].rearrange("p (c b) -> p c b", c=KH),
                in0=stv[:, :, t * BC:(t + 1) * BC],
                in1=n[:].rearrange("p (c b) -> p c b", c=KH),
                op=OP.subtract)
            e_ = tmp("e")
            nc.vector.tensor_tensor(out=e_[:], in0=rz[:, KH * BC:2 * KH * BC],
                                    in1=d[:], op=OP.mult)
            nc.vector.tensor_tensor(
                out=stv[:, :, (t + 1) * BC:(t + 2) * BC],
                in0=n[:].rearrange("p (c b) -> p c b", c=KH),
                in1=e_[:].rearrange("p (c b) -> p c b", c=KH), op=OP.add)

        for th in stage_x0(0):
            th()
        for c in range(NCH):
            pend = stage_x0(c + 1) if c + 1 < NCH else []
            if c >= 1:
                stage_x1(c - 1)
            for ts in range(CH):
                step(0, c * CH + ts)
                if pend and ts % 2 == 1:
                    pend.pop(0)()
                if c >= 1:
                    step(1, (c - 1) * CH + ts)
            for th in pend:
                th()
        stage_x1(NCH - 1)
        for ts in range(CH):
            step(1, (NCH - 1) * CH + ts)

        nc.vector.tensor_reduce(
            out=pooled[:].rearrange("p (c t) -> p c t", c=KH),
            in_=st1[:].rearrange("p (c s b) -> p c s b", c=KH, b=BC)[:, :, 1:T + 1, :],
            axis=mybir.AxisListType.X, op=OP.add)
        nc.sync.dma_start(out=out_d[:], in_=pooled[:])

    nc.compile()
    return nc


def _prep(inputs):
    bf = ml_dtypes.bfloat16

    def packT(W):
        WT = np.ascontiguousarray(W.T).astype(np.float32)
        K = WT.shape[0]
        nk = (K + 127) // 128
        pad = np.zeros((nk * 128, G3), np.float32)
        pad[:K] = WT
        return np.concatenate([pad[k * 128:(k + 1) * 128] for k in range(nk)],
                              axis=1).astype(bf)

    def bias_rz(bih, bhh):
        b = bih.astype(np.float32).copy()
        b[:2 * H] += bhh[:2 * H].astype(np.float32)
        return np.ascontiguousarray(b.reshape(M3, 128).T)

    def bias_n(bhh):
        bn = np.ascontiguousarray(
            bhh[2 * H:].astype(np.float32).reshape(KH, 128).T)
        return np.repeat(bn, BC, axis=1)

    common = {
        "emb": np.ascontiguousarray(inputs["emb"], dtype=np.float32),
        "w0i": packT(inputs["Wih0"]),
        "w0h": packT(inputs["Whh0"]),
        "w1i": packT(inputs["Wih1"]),
        "w1h": packT(inputs["Whh1"]),
        "b0": bias_rz(inputs["bih0"], inputs["bhh0"]),
        "b1": bias_rz(inputs["bih1"], inputs["bhh1"]),
        "bn0": bias_n(inputs["bhh0"]),
        "bn1": bias_n(inputs["bhh1"]),
    }
    texts = np.asarray(inputs["texts"])
    in_maps = []
    for c in range(NCORE):
        idxc = np.ascontiguousarray(
            texts[:, c * BC:(c + 1) * BC].astype(np.int32)
            .reshape(NBLK, 128).T)
        in_maps.append({**common, "idx": idxc})
    return in_maps


def _postproc(results, inputs):
    s = np.zeros((128, KH * T), np.float32)
    for r in results:
        s += r["out"]
    pooled = s.reshape(128, KH, T).transpose(2, 1, 0).reshape(T, H) / B
    fc_W = np.asarray(inputs["fc_W"], dtype=np.float32)
    fc_b = np.asarray(inputs["fc_b"], dtype=np.float32)
    return (pooled @ fc_W.T + fc_b).astype(np.float32)


def kernel(**inputs):
    from concourse import bass_utils
    if "nc" not in _cache:
        _cache["nc"] = _build()
    nc = _cache["nc"]
    in_maps = _prep(inputs)
    res = bass_utils.run_bass_kernel_spmd(
        nc, in_maps, core_ids=list(range(NCORE)))
    return _postproc([res.results[i] for i in range(NCORE)], inputs)


if __name__ == "__main__":
    import time
    t0 = time.time()
    nc = _build()
    print("build+compile time:", round(time.time() - t0, 1), "s")
